# revision 7
# baseline (speedup 1.0000x reference)
"""Trainium2 Bass kernel for nn_Attention (pooling attention).

Math (per batch b):
    u[b]     = W_score @ h_t[b]            (score = (hidden @ W_score) . h_t
                                            collapses to hidden . (W_score @ h_t))
    score[t] = hidden[b,t,:] . u[b]        (DVE fp16 mul + pairwise-add tree)
    p[t]     = exp(score[t] - 50)          (ScalarE -> bf16, fused accum -> q)
    s        = sum_t p[t]                  (PE ones-matmul over q)
    ctx      = (sum_t p[t] * hidden[b,t,:]) / s
               (PE: bf16 p column as 1-col stationary vs fp16 y; the 1/s
                normalization folds into the PSUM->SBUF copy via ACT scale)
    out[b]   = tanh([ctx, h_t[b]] @ W_att)

bf16 p is overflow-safe (fp32-range exponent), so the context matmuls start
right after exp -- the softmax denominator (s -> 1/s) is computed concurrently
and only gates the tiny ctx_row copy.  For the last batch this goes further:
exp and the context matmuls run per quarter-chunk as the chunked loads land.

Sharding: data-parallel over batch, 16 batches per core on 8 cores; weights
replicated.  hidden_states is read from HBM exactly once (fp32), cast to fp16
during the DMA (SWDGE cast), and never transposed.

Pipeline design:
  - The y16 load flood starts immediately; the small setup DMAs (ident, ht,
    wst, watt) ride the sync HWDGE ring concurrently.  No other DMA is issued
    during the flood window (tiny DMAs inside the flood see ~20us latency).
  - All u[b]/broadcast work happens on PE+ACT only, in fp16, and is emitted
    interleaved with the batch loop so neither the PE nor ACT FIFO ever
    head-of-line blocks the per-batch critical chain.
  - softmax sum via PE ones-matmul; reciprocal on DVE at the END of the next
    batch's op stream (fully latency-covered).
"""

import sys

import numpy as np

_TRN_REPO = "/opt/trn_rl_repo"
if _TRN_REPO not in sys.path:
    sys.path.insert(0, _TRN_REPO)

import concourse.bass as bass
import concourse.bacc as bacc
import concourse.tile as tile
from concourse import mybir
from concourse.bass_utils import run_bass_kernel_spmd

N_CORES = 8
B, T, H = 128, 2048, 256
NB = B // N_CORES  # batches per core
P = 128  # SBUF partitions
TT = T // P  # t-tiles per batch
OUT_D = 128
EXP_SHIFT = -50.0  # keeps exp() in fp32/bf16 range; cancels in the softmax ratio

NCH = 4  # last batch is loaded/scored in NCH chunks to shorten the tail
CTT = TT // NCH
UPRE = 4  # u-chains emitted before the loop; chain b+UPRE emitted in iter b

F32 = mybir.dt.float32
F16 = mybir.dt.float16
BF16 = mybir.dt.bfloat16


def _build_kernel(nc: bass.Bass, tc: "tile.TileContext", hidden, wst, watt, ident, out):
    add = mybir.AluOpType.add

    from contextlib import ExitStack

    with ExitStack() as ctx:
        const = ctx.enter_context(tc.tile_pool(name="const", bufs=1))
        ybufs = ctx.enter_context(tc.tile_pool(name="ybufs", bufs=8))
        sc = ctx.enter_context(tc.tile_pool(name="sc", bufs=3))
        psum_t = ctx.enter_context(tc.tile_pool(name="psum_t", bufs=2, space="PSUM"))
        psum_u = ctx.enter_context(tc.tile_pool(name="psum_u", bufs=2, space="PSUM"))
        psum_p = ctx.enter_context(tc.tile_pool(name="psum_p", bufs=1, space="PSUM"))

        # ---- constants (no DMA needed) -------------------------------------
        ones_row16 = const.tile([1, P], F16, tag="ones_row16")
        nc.vector.memset(ones_row16, 1.0)
        ones128 = const.tile([P, P], F32, tag="ones128")
        nc.vector.memset(ones128, 1.0)
        ones_col1 = const.tile([1, 1], F32, tag="ones_col1")
        nc.vector.memset(ones_col1, 1.0)
        shift_col = const.tile([P, 1], F32, tag="shift_col")
        nc.vector.memset(shift_col, EXP_SHIFT)

        # ---- setup DMAs: all independent, all on the sync HWDGE ring -------
        ident_sb = const.tile([16, 16], F32, tag="ident")
        nc.sync.dma_start(out=ident_sb, in_=ident[:, :])
        ht_sb = const.tile([NB, H], F32, tag="ht")
        nc.sync.dma_start(out=ht_sb, in_=hidden[:, T - 1, :])
        wst_sb = const.tile([P, 2, H], F32, tag="wst")  # W_score^T as [k, kk, h]
        nc.sync.dma_start(out=wst_sb, in_=wst.rearrange("(kk p) h -> p kk h", p=P))
        watt_sb = const.tile([P, 4, OUT_D], F32, tag="watt")  # W_att as [d, dd, j]
        nc.sync.dma_start(out=watt_sb, in_=watt.rearrange("(dd p) j -> p dd j", p=P))

        # ---- y16 load flood (SWDGE cast fp32->fp16), starts immediately ----
        ylist = {}
        for k in range(NB - 1):
            y = ybufs.tile([P, TT, H], F16, tag="y16", name=f"y16_{k}")
            nc.gpsimd.dma_start(
                out=y, in_=hidden[k].rearrange("(p i) h -> p i h", i=TT)
            )
            ylist[k] = y
        ychunks = []
        hlast = hidden[NB - 1].rearrange("(p i) h -> p i h", i=TT)
        for c in range(NCH):
            yc = ybufs.tile([P, CTT, H], F16, tag="y16c", name=f"y16c_{c}")
            nc.gpsimd.dma_start(out=yc, in_=hlast[:, c * CTT : (c + 1) * CTT, :])
            ychunks.append(yc)

        # ---- h_t^T (fp16) and fp16 copies of the weights -------------------
        htT16 = const.tile([P, 2, NB], F16, tag="htT16")  # h_t^T halves [k, half, b]
        for half in range(2):
            ps_tr = psum_t.tile([P, NB], F32, tag="ptmp", name=f"ps_tr{half}")
            nc.tensor.matmul(
                ps_tr,
                lhsT=ht_sb[:, half * P : (half + 1) * P],
                rhs=ident_sb,
                start=True,
                stop=True,
            )
            nc.scalar.copy(out=htT16[:, half, :], in_=ps_tr)
        wst16 = const.tile([P, 2, H], F16, tag="wst16")
        nc.scalar.copy(out=wst16, in_=wst_sb)
        watt16 = const.tile([P, 4, OUT_D], F16, tag="watt16")
        nc.scalar.copy(out=watt16, in_=watt_sb)

        # u[b] = h_t[b] @ W_score^T via M=1 fp16 matmuls (keeps everything at
        # partition 0); then broadcast to all 128 partitions via a K=1
        # matmul.  No DMAs -> nothing contends with the flood.
        ubc_all = const.tile([P, NB, H], F16, tag="ubc_all")

        def emit_uchain(b):
            ps_ub = psum_u.tile([1, H], F32, tag="pub", name=f"pub{b}")
            for half in range(2):
                nc.tensor.matmul(
                    ps_ub,
                    lhsT=htT16[:, half, b : b + 1],
                    rhs=wst16[:, half, :],
                    start=(half == 0),
                    stop=(half == 1),
                )
            u16b = sc.tile([1, H], F16, tag="u16b", name=f"u16b{b}")
            nc.scalar.copy(out=u16b, in_=ps_ub)
            ps_ubc = psum_t.tile([P, H], F32, tag="ptmp", name=f"pubc{b}")
            nc.tensor.matmul(ps_ubc, lhsT=ones_row16, rhs=u16b, start=True, stop=True)
            nc.scalar.copy(out=ubc_all[:, b, :], in_=ps_ubc)

        for b in range(UPRE):
            emit_uchain(b)

        # ---- persistent PSUM accumulators for ctx^T ------------------------
        ctxT_ps = [
            psum_p.tile([P, NB], F32, tag=f"ctxT{j}", name=f"ctxT{j}")
            for j in range(2)
        ]

        # ---- per-batch score pipeline --------------------------------------
        # t = p*TT + i block mapping gives 16KB-contiguous DMA runs per
        # partition (softmax/context are t-permutation-invariant).
        state = {}  # batch -> dict of tiles needed by the deferred stages

        def ubc_rep(b, rep):
            ubc = ubc_all[:, b, :]
            return bass.AP(
                tensor=ubc.tensor,
                offset=ubc.offset,
                ap=[list(ubc.ap[0]), [0, rep], list(ubc.ap[1])],
            )

        def emit_score_tail(b, z, score, nt):
            z1 = sc.tile([P, nt, 128], F16, tag="z1" if nt == TT else "z1c")
            nc.vector.tensor_add(z1, z[:, :, 0:128], z[:, :, 128:256])
            z2 = sc.tile([P, nt, 64], F16, tag="z2" if nt == TT else "z2c")
            nc.vector.tensor_add(z2, z1[:, :, 0:64], z1[:, :, 64:128])
            nc.vector.tensor_reduce(
                out=score, in_=z2, axis=mybir.AxisListType.X, op=add
            )

        def emit_exp(b, score, p_t, q):
            # p in bf16: overflow-safe unnormalized weights, feeds PE directly
            nc.scalar.activation(
                out=p_t,
                in_=score,
                func=mybir.ActivationFunctionType.Exp,
                bias=shift_col,
                scale=1.0,
                accum_out=q,
            )

        def emit_s(b, q):
            s_ps = psum_u.tile([P, 1], F32, tag="pub", name=f"s{b}")
            nc.tensor.matmul(s_ps, lhsT=ones128, rhs=q, start=True, stop=True)
            state[b]["s_ps"] = s_ps

        def emit_recip(b):
            rs = sc.tile([P, 1], F32, tag="rs")
            nc.vector.reciprocal(out=rs, in_=state[b]["s_ps"])
            state[b]["rs"] = rs

        def emit_ctx(b):
            p_t = state[b]["p"]
            ctx_ps = psum_t.tile([1, H], F32, tag="ptmp", name=f"ctx{b}")
            y16 = ylist.pop(b)
            for i in range(TT):
                nc.tensor.matmul(
                    ctx_ps,
                    lhsT=p_t[:, i : i + 1],
                    rhs=y16[:, i, :],
                    start=(i == 0),
                    stop=(i == TT - 1),
                )
            state[b]["ctx_ps"] = ctx_ps

        def emit_ctx_row(b):
            # normalization by 1/s happens here, on the [1, 256] row only
            ctx_row = sc.tile([1, H], F32, tag="ctx_row")
            nc.scalar.mul(
                out=ctx_row, in_=state[b]["ctx_ps"], mul=state[b]["rs"][0:1, :]
            )
            state[b]["ctx_row"] = ctx_row

        def emit_scatter(b):
            ctx_row = state[b]["ctx_row"]
            for j in range(2):
                nc.tensor.matmul(
                    ctxT_ps[j][:, b : b + 1],
                    lhsT=ctx_row[:, j * P : (j + 1) * P],
                    rhs=ones_col1,
                    start=True,
                    stop=True,
                )
            del state[b]

        for b in range(NB - 1):
            if b + UPRE < NB:
                emit_uchain(b + UPRE)
            z = sc.tile([P, TT, H], F16, tag="z")
            nc.vector.tensor_mul(z, ylist[b], ubc_rep(b, TT))
            if b >= 1:
                emit_ctx(b - 1)
            score = sc.tile([P, TT], F32, tag="score")
            emit_score_tail(b, z, score, TT)
            if b >= 1:
                emit_recip(b - 1)  # end of DVE stream: exp/sum latency covered
            p_t = sc.tile([P, TT], BF16, tag="p")
            q = sc.tile([P, 1], F32, tag="q")
            state[b] = {"p": p_t}
            emit_exp(b, score, p_t, q)
            emit_s(b, q)
            if b >= 1:
                emit_ctx_row(b - 1)
                emit_scatter(b - 1)

        # ---- last batch: per-chunk exp + ctx (denominator deferred) --------
        bl = NB - 1
        score = sc.tile([P, TT], F32, tag="score")
        p_t = sc.tile([P, TT], BF16, tag="p")
        q4 = sc.tile([P, NCH], F32, tag="q4")
        ctxl_ps = psum_t.tile([1, H], F32, tag="ptmp", name="ctxl")
        state[bl] = {"p": p_t}
        for c in range(NCH):
            zc = sc.tile([P, CTT, H], F16, tag="zc")
            nc.vector.tensor_mul(zc, ychunks[c], ubc_rep(bl, CTT))
            if c == 0:
                emit_ctx(bl - 1)
            sl = slice(c * CTT, (c + 1) * CTT)
            emit_score_tail(bl, zc, score[:, sl], CTT)
            if c == 0:
                emit_recip(bl - 1)
                emit_ctx_row(bl - 1)
                emit_scatter(bl - 1)
            emit_exp(bl, score[:, sl], p_t[:, sl], q4[:, c : c + 1])
            for i in range(c * CTT, (c + 1) * CTT):
                nc.tensor.matmul(
                    ctxl_ps,
                    lhsT=p_t[:, i : i + 1],
                    rhs=ychunks[c][:, i % CTT, :],
                    start=(i == 0),
                    stop=(i == TT - 1),
                )
        state[bl]["ctx_ps"] = ctxl_ps
        s4_ps = psum_u.tile([P, NCH], F32, tag="pub", name="s4")
        nc.tensor.matmul(s4_ps, lhsT=ones128, rhs=q4, start=True, stop=True)
        s_tot = sc.tile([P, 1], F32, tag="rs", name="s_tot")
        nc.vector.tensor_reduce(out=s_tot, in_=s4_ps, axis=mybir.AxisListType.X, op=add)
        rs = sc.tile([P, 1], F32, tag="rs", name="rs_l")
        nc.vector.reciprocal(out=rs, in_=s_tot)
        state[bl]["rs"] = rs
        emit_ctx_row(bl)
        emit_scatter(bl)

        # ---- finalize: concat with h_t, @W_att, tanh -----------------------
        preT = sc.tile([P, 2, NB], F16, tag="preT")
        for j in range(2):
            nc.scalar.copy(out=preT[:, j, :], in_=ctxT_ps[j])

        out_ps = psum_t.tile([NB, OUT_D], F32, tag="ptmp")
        for dd in range(4):
            lhsT = preT[:, dd, :] if dd < 2 else htT16[:, dd - 2, :]
            nc.tensor.matmul(
                out_ps,
                lhsT=lhsT,
                rhs=watt16[:, dd, :],
                start=(dd == 0),
                stop=(dd == 3),
            )
        out_sb = sc.tile([NB, OUT_D], F32, tag="out_sb")
        nc.scalar.activation(
            out=out_sb, in_=out_ps, func=mybir.ActivationFunctionType.Tanh
        )
        nc.sync.dma_start(out=out[:, :], in_=out_sb)


_NC_CACHE = {}


def _get_nc():
    if "nc" not in _NC_CACHE:
        nc = bacc.Bacc("TRN2", target_bir_lowering=False, debug=False)
        hidden = nc.declare_dram_parameter("hidden", [NB, T, H], F32, isOutput=False)
        wst = nc.declare_dram_parameter("w_score_t", [H, H], F32, isOutput=False)
        watt = nc.declare_dram_parameter("w_att", [2 * H, OUT_D], F32, isOutput=False)
        ident = nc.declare_dram_parameter("ident16", [16, 16], F32, isOutput=False)
        out = nc.declare_dram_parameter("out", [NB, OUT_D], F32, isOutput=True)
        with tile.TileContext(nc) as tc:
            _build_kernel(nc, tc, hidden, wst, watt, ident, out)
        nc.compile()
        _NC_CACHE["nc"] = nc
    return _NC_CACHE["nc"]


def _run(hidden_states, W_score, W_att, trace=False, trace_kwargs=None):
    hidden_states = np.ascontiguousarray(np.asarray(hidden_states, dtype=np.float32))
    W_score = np.asarray(W_score, dtype=np.float32)
    W_att = np.ascontiguousarray(np.asarray(W_att, dtype=np.float32))
    wst = np.ascontiguousarray(W_score.T)
    ident = np.eye(16, dtype=np.float32)

    nc = _get_nc()
    in_maps = []
    for c in range(N_CORES):
        in_maps.append(
            {
                "hidden": hidden_states[c * NB : (c + 1) * NB],
                "w_score_t": wst,
                "w_att": W_att,
                "ident16": ident,
            }
        )
    kwargs = {}
    if trace:
        kwargs["trace"] = True
        if trace_kwargs:
            kwargs.update(trace_kwargs)
    res = run_bass_kernel_spmd(nc, in_maps, list(range(N_CORES)), **kwargs)
    out = np.concatenate([res.results[c]["out"] for c in range(N_CORES)], axis=0)
    return out, res


def kernel(hidden_states, W_score, W_att):
    out, _ = _run(hidden_states, W_score, W_att, trace=False)
    return out


# revision 10
# speedup vs baseline: 1.0810x; 1.0810x over previous
"""Trainium2 Bass kernel for nn_Attention (pooling attention).

Math (per batch b):
    u[b]     = W_score @ h_t[b]            (score = (hidden @ W_score) . h_t
                                            collapses to hidden . (W_score @ h_t))
    score[t] = hidden[b,t,:] . u[b]        (DVE fp16 mul + pairwise-add tree)
    p[t]     = exp(score[t] - 50)          (ScalarE -> bf16, fused accum -> q)
    s        = sum_t p[t]                  (PE ones-matmul over q)
    ctx      = (sum_t p[t] * hidden[b,t,:]) / s
               (PE: bf16 p column as 1-col stationary vs fp16 y; the 1/s
                normalization folds into the PSUM->SBUF copy via ACT scale)
    out[b]   = tanh([ctx, h_t[b]] @ W_att)

bf16 p is overflow-safe (fp32-range exponent), so the context matmuls start
right after exp -- the softmax denominator (s -> 1/s) is computed concurrently
and only gates the tiny ctx_row copy.  For the last batch this goes further:
exp and the context matmuls run per quarter-chunk as the chunked loads land.

Sharding: data-parallel over batch, 16 batches per core on 8 cores; weights
replicated.  hidden_states is read from HBM exactly once (fp32), cast to fp16
during the DMA (SWDGE cast), and never transposed.

Pipeline design:
  - The y16 load flood starts immediately; the small setup DMAs (ident, ht,
    wst, watt) ride the sync HWDGE ring concurrently.  No other DMA is issued
    during the flood window (tiny DMAs inside the flood see ~20us latency).
  - All u[b]/broadcast work happens on PE+ACT only, in fp16, and is emitted
    interleaved with the batch loop so neither the PE nor ACT FIFO ever
    head-of-line blocks the per-batch critical chain.
  - softmax sum via PE ones-matmul; reciprocal on DVE at the END of the next
    batch's op stream (fully latency-covered).
"""

import sys

import numpy as np

_TRN_REPO = "/opt/trn_rl_repo"
if _TRN_REPO not in sys.path:
    sys.path.insert(0, _TRN_REPO)

import concourse.bass as bass
import concourse.bacc as bacc
import concourse.tile as tile
from concourse import mybir
from concourse.bass_utils import run_bass_kernel_spmd

N_CORES = 8
B, T, H = 128, 2048, 256
NB = B // N_CORES  # batches per core
P = 128  # SBUF partitions
TT = T // P  # t-tiles per batch
OUT_D = 128
EXP_SHIFT = -50.0  # keeps exp() in fp32/bf16 range; cancels in the softmax ratio

NCH = 4  # last batch is loaded/scored in NCH chunks to shorten the tail
CTT = TT // NCH
UPRE = 4  # u-chains emitted before the loop; chain b+UPRE emitted in iter b

F32 = mybir.dt.float32
F16 = mybir.dt.float16
BF16 = mybir.dt.bfloat16


def _build_kernel(nc: bass.Bass, tc: "tile.TileContext", hidden, wst, watt, ident, out):
    add = mybir.AluOpType.add

    from contextlib import ExitStack

    with ExitStack() as ctx:
        const = ctx.enter_context(tc.tile_pool(name="const", bufs=1))
        ybufs = ctx.enter_context(tc.tile_pool(name="ybufs", bufs=8))
        sc = ctx.enter_context(tc.tile_pool(name="sc", bufs=3))
        psum_t = ctx.enter_context(tc.tile_pool(name="psum_t", bufs=2, space="PSUM"))
        psum_u = ctx.enter_context(tc.tile_pool(name="psum_u", bufs=2, space="PSUM"))
        psum_p = ctx.enter_context(tc.tile_pool(name="psum_p", bufs=1, space="PSUM"))

        # ---- constants (no DMA needed) -------------------------------------
        ones_row16 = const.tile([1, P], F16, tag="ones_row16")
        nc.vector.memset(ones_row16, 1.0)
        ones128 = const.tile([P, P], F32, tag="ones128")
        nc.vector.memset(ones128, 1.0)
        ones_col1 = const.tile([1, 1], F32, tag="ones_col1")
        nc.vector.memset(ones_col1, 1.0)
        shift_col = const.tile([P, 1], F32, tag="shift_col")
        nc.vector.memset(shift_col, EXP_SHIFT)

        # ---- setup DMAs ----------------------------------------------------
        # ident/ht/wst ride the SWDGE queue AHEAD of the y16 flood: they
        # complete in ring order (~10us) instead of dribbling through the
        # SDMA round-robin behind the flood (measured 12..40us on the sync
        # ring).  watt is only needed by the epilogue, so it may dribble.
        ident_sb = const.tile([16, 16], F32, tag="ident")
        nc.gpsimd.dma_start(out=ident_sb, in_=ident[:, :])
        ht_sb = const.tile([NB, H], F32, tag="ht")
        nc.gpsimd.dma_start(out=ht_sb, in_=hidden[:, T - 1, :])
        wst_sb = const.tile([P, 2, H], F32, tag="wst")  # W_score^T as [k, kk, h]
        nc.gpsimd.dma_start(out=wst_sb, in_=wst.rearrange("(kk p) h -> p kk h", p=P))
        watt_sb = const.tile([P, 4, OUT_D], F32, tag="watt")  # W_att as [d, dd, j]
        nc.sync.dma_start(out=watt_sb, in_=watt.rearrange("(dd p) j -> p dd j", p=P))

        # ---- y16 load flood (SWDGE cast fp32->fp16), starts immediately ----
        ylist = {}
        for k in range(NB - 1):
            y = ybufs.tile([P, TT, H], F16, tag="y16", name=f"y16_{k}")
            nc.gpsimd.dma_start(
                out=y, in_=hidden[k].rearrange("(p i) h -> p i h", i=TT)
            )
            ylist[k] = y
        ychunks = []
        hlast = hidden[NB - 1].rearrange("(p i) h -> p i h", i=TT)
        for c in range(NCH):
            yc = ybufs.tile([P, CTT, H], F16, tag="y16c", name=f"y16c_{c}")
            nc.gpsimd.dma_start(out=yc, in_=hlast[:, c * CTT : (c + 1) * CTT, :])
            ychunks.append(yc)

        # ---- h_t^T (fp16) and fp16 copies of the weights -------------------
        htT16 = const.tile([P, 2, NB], F16, tag="htT16")  # h_t^T halves [k, half, b]
        for half in range(2):
            ps_tr = psum_t.tile([P, NB], F32, tag="ptmp", name=f"ps_tr{half}")
            nc.tensor.matmul(
                ps_tr,
                lhsT=ht_sb[:, half * P : (half + 1) * P],
                rhs=ident_sb,
                start=True,
                stop=True,
            )
            nc.scalar.copy(out=htT16[:, half, :], in_=ps_tr)
        wst16 = const.tile([P, 2, H], F16, tag="wst16")
        nc.scalar.copy(out=wst16, in_=wst_sb)

        # u[b] = h_t[b] @ W_score^T via M=1 fp16 matmuls (keeps everything at
        # partition 0); then broadcast to all 128 partitions via a K=1
        # matmul.  No DMAs -> nothing contends with the flood.
        ubc_all = const.tile([P, NB, H], F16, tag="ubc_all")

        def emit_uchain(b):
            ps_ub = psum_u.tile([1, H], F32, tag="pub", name=f"pub{b}")
            for half in range(2):
                nc.tensor.matmul(
                    ps_ub,
                    lhsT=htT16[:, half, b : b + 1],
                    rhs=wst16[:, half, :],
                    start=(half == 0),
                    stop=(half == 1),
                )
            u16b = sc.tile([1, H], F16, tag="u16b", name=f"u16b{b}")
            nc.scalar.copy(out=u16b, in_=ps_ub)
            ps_ubc = psum_t.tile([P, H], F32, tag="ptmp", name=f"pubc{b}")
            nc.tensor.matmul(ps_ubc, lhsT=ones_row16, rhs=u16b, start=True, stop=True)
            nc.scalar.copy(out=ubc_all[:, b, :], in_=ps_ubc)

        for b in range(UPRE):
            emit_uchain(b)

        # ---- persistent PSUM accumulators for ctx^T ------------------------
        ctxT_ps = [
            psum_p.tile([P, NB], F32, tag=f"ctxT{j}", name=f"ctxT{j}")
            for j in range(2)
        ]

        # ---- per-batch score pipeline --------------------------------------
        # t = p*TT + i block mapping gives 16KB-contiguous DMA runs per
        # partition (softmax/context are t-permutation-invariant).
        state = {}  # batch -> dict of tiles needed by the deferred stages

        def ubc_rep(b, rep):
            ubc = ubc_all[:, b, :]
            return bass.AP(
                tensor=ubc.tensor,
                offset=ubc.offset,
                ap=[list(ubc.ap[0]), [0, rep], list(ubc.ap[1])],
            )

        def emit_score_tail(b, z, score, nt):
            z1 = sc.tile([P, nt, 128], F16, tag="z1" if nt == TT else "z1c")
            nc.vector.tensor_add(z1, z[:, :, 0:128], z[:, :, 128:256])
            z2 = sc.tile([P, nt, 64], F16, tag="z2" if nt == TT else "z2c")
            nc.vector.tensor_add(z2, z1[:, :, 0:64], z1[:, :, 64:128])
            nc.vector.tensor_reduce(
                out=score, in_=z2, axis=mybir.AxisListType.X, op=add
            )

        def emit_exp(b, score, p_t, q):
            # p in bf16: overflow-safe unnormalized weights, feeds PE directly
            nc.scalar.activation(
                out=p_t,
                in_=score,
                func=mybir.ActivationFunctionType.Exp,
                bias=shift_col,
                scale=1.0,
                accum_out=q,
            )

        def emit_s(b, q):
            s_ps = psum_u.tile([P, 1], F32, tag="pub", name=f"s{b}")
            nc.tensor.matmul(s_ps, lhsT=ones128, rhs=q, start=True, stop=True)
            state[b]["s_ps"] = s_ps

        def emit_recip(b):
            rs = sc.tile([P, 1], F32, tag="rs")
            nc.vector.reciprocal(out=rs, in_=state[b]["s_ps"])
            state[b]["rs"] = rs

        def emit_ctx(b):
            p_t = state[b]["p"]
            ctx_ps = psum_t.tile([1, H], F32, tag="ptmp", name=f"ctx{b}")
            y16 = ylist.pop(b)
            for i in range(TT):
                nc.tensor.matmul(
                    ctx_ps,
                    lhsT=p_t[:, i : i + 1],
                    rhs=y16[:, i, :],
                    start=(i == 0),
                    stop=(i == TT - 1),
                )
            state[b]["ctx_ps"] = ctx_ps

        def emit_ctx_row(b):
            # normalization by 1/s happens here, on the [1, 256] row only
            ctx_row = sc.tile([1, H], F32, tag="ctx_row")
            nc.scalar.mul(
                out=ctx_row, in_=state[b]["ctx_ps"], mul=state[b]["rs"][0:1, :]
            )
            state[b]["ctx_row"] = ctx_row

        def emit_scatter(b):
            ctx_row = state[b]["ctx_row"]
            for j in range(2):
                nc.tensor.matmul(
                    ctxT_ps[j][:, b : b + 1],
                    lhsT=ctx_row[:, j * P : (j + 1) * P],
                    rhs=ones_col1,
                    start=True,
                    stop=True,
                )
            del state[b]

        for b in range(NB - 1):
            if b + UPRE < NB:
                emit_uchain(b + UPRE)
            z = sc.tile([P, TT, H], F16, tag="z")
            nc.vector.tensor_mul(z, ylist[b], ubc_rep(b, TT))
            if b >= 1:
                emit_ctx(b - 1)
            score = sc.tile([P, TT], F32, tag="score")
            emit_score_tail(b, z, score, TT)
            if b >= 1:
                emit_recip(b - 1)  # end of DVE stream: exp/sum latency covered
            p_t = sc.tile([P, TT], BF16, tag="p")
            q = sc.tile([P, 1], F32, tag="q")
            state[b] = {"p": p_t}
            emit_exp(b, score, p_t, q)
            emit_s(b, q)
            if b >= 1:
                emit_ctx_row(b - 1)
                emit_scatter(b - 1)

        # ---- last batch: per-chunk exp + ctx (denominator deferred) --------
        bl = NB - 1
        score = sc.tile([P, TT], F32, tag="score")
        p_t = sc.tile([P, TT], BF16, tag="p")
        q4 = sc.tile([P, NCH], F32, tag="q4")
        ctxl_ps = psum_t.tile([1, H], F32, tag="ptmp", name="ctxl")
        state[bl] = {"p": p_t}
        for c in range(NCH):
            zc = sc.tile([P, CTT, H], F16, tag="zc")
            nc.vector.tensor_mul(zc, ychunks[c], ubc_rep(bl, CTT))
            if c == 0:
                emit_ctx(bl - 1)
            sl = slice(c * CTT, (c + 1) * CTT)
            emit_score_tail(bl, zc, score[:, sl], CTT)
            if c == 0:
                emit_recip(bl - 1)
                emit_ctx_row(bl - 1)
                emit_scatter(bl - 1)
            emit_exp(bl, score[:, sl], p_t[:, sl], q4[:, c : c + 1])
            for i in range(c * CTT, (c + 1) * CTT):
                nc.tensor.matmul(
                    ctxl_ps,
                    lhsT=p_t[:, i : i + 1],
                    rhs=ychunks[c][:, i % CTT, :],
                    start=(i == 0),
                    stop=(i == TT - 1),
                )
        state[bl]["ctx_ps"] = ctxl_ps
        s4_ps = psum_u.tile([P, NCH], F32, tag="pub", name="s4")
        nc.tensor.matmul(s4_ps, lhsT=ones128, rhs=q4, start=True, stop=True)
        s_tot = sc.tile([P, 1], F32, tag="rs", name="s_tot")
        nc.vector.tensor_reduce(out=s_tot, in_=s4_ps, axis=mybir.AxisListType.X, op=add)
        rs = sc.tile([P, 1], F32, tag="rs", name="rs_l")
        nc.vector.reciprocal(out=rs, in_=s_tot)
        state[bl]["rs"] = rs
        emit_ctx_row(bl)
        emit_scatter(bl)

        # ---- finalize: concat with h_t, @W_att, tanh -----------------------
        # watt16 cast sits here so its wait on the (slow, sync-ring) watt DMA
        # never head-of-line blocks the per-batch ACT stream
        watt16 = const.tile([P, 4, OUT_D], F16, tag="watt16")
        nc.scalar.copy(out=watt16, in_=watt_sb)
        preT = sc.tile([P, 2, NB], F16, tag="preT")
        for j in range(2):
            nc.scalar.copy(out=preT[:, j, :], in_=ctxT_ps[j])

        out_ps = psum_t.tile([NB, OUT_D], F32, tag="ptmp")
        for dd in range(4):
            lhsT = preT[:, dd, :] if dd < 2 else htT16[:, dd - 2, :]
            nc.tensor.matmul(
                out_ps,
                lhsT=lhsT,
                rhs=watt16[:, dd, :],
                start=(dd == 0),
                stop=(dd == 3),
            )
        out_sb = sc.tile([NB, OUT_D], F32, tag="out_sb")
        nc.scalar.activation(
            out=out_sb, in_=out_ps, func=mybir.ActivationFunctionType.Tanh
        )
        nc.sync.dma_start(out=out[:, :], in_=out_sb)


_NC_CACHE = {}


def _get_nc():
    if "nc" not in _NC_CACHE:
        nc = bacc.Bacc("TRN2", target_bir_lowering=False, debug=False)
        hidden = nc.declare_dram_parameter("hidden", [NB, T, H], F32, isOutput=False)
        wst = nc.declare_dram_parameter("w_score_t", [H, H], F32, isOutput=False)
        watt = nc.declare_dram_parameter("w_att", [2 * H, OUT_D], F32, isOutput=False)
        ident = nc.declare_dram_parameter("ident16", [16, 16], F32, isOutput=False)
        out = nc.declare_dram_parameter("out", [NB, OUT_D], F32, isOutput=True)
        with tile.TileContext(nc) as tc:
            _build_kernel(nc, tc, hidden, wst, watt, ident, out)
        nc.compile()
        _NC_CACHE["nc"] = nc
    return _NC_CACHE["nc"]


def _run(hidden_states, W_score, W_att, trace=False, trace_kwargs=None):
    hidden_states = np.ascontiguousarray(np.asarray(hidden_states, dtype=np.float32))
    W_score = np.asarray(W_score, dtype=np.float32)
    W_att = np.ascontiguousarray(np.asarray(W_att, dtype=np.float32))
    wst = np.ascontiguousarray(W_score.T)
    ident = np.eye(16, dtype=np.float32)

    nc = _get_nc()
    in_maps = []
    for c in range(N_CORES):
        in_maps.append(
            {
                "hidden": hidden_states[c * NB : (c + 1) * NB],
                "w_score_t": wst,
                "w_att": W_att,
                "ident16": ident,
            }
        )
    kwargs = {}
    if trace:
        kwargs["trace"] = True
        if trace_kwargs:
            kwargs.update(trace_kwargs)
    res = run_bass_kernel_spmd(nc, in_maps, list(range(N_CORES)), **kwargs)
    out = np.concatenate([res.results[c]["out"] for c in range(N_CORES)], axis=0)
    return out, res


def kernel(hidden_states, W_score, W_att):
    out, _ = _run(hidden_states, W_score, W_att, trace=False)
    return out


# revision 12
# speedup vs baseline: 1.1502x; 1.0640x over previous
"""Trainium2 Bass kernel for nn_Attention (pooling attention).

Math (per batch b):
    u[b]     = W_score @ h_t[b]            (score = (hidden @ W_score) . h_t
                                            collapses to hidden . (W_score @ h_t))
    score[t] = hidden[b,t,:] . u[b]        (DVE fp16 mul + split reduction:
                                            10 t-tiles via DVE pairwise tree,
                                            6 t-tiles via ACT copy-with-accum)
    p[t]     = exp(score[t] - 50)          (ScalarE -> bf16, fused accum -> q)
    s        = sum_t p[t]                  (PE ones-matmul over q)
    ctx      = (sum_t p[t] * hidden[b,t,:]) / s
               (PE: bf16 p column as 1-col stationary vs fp16 y; the 1/s
                normalization folds into the PSUM->SBUF copy via ACT scale)
    out[b]   = tanh([ctx, h_t[b]] @ W_att)

bf16 p is overflow-safe (fp32-range exponent), so the context matmuls start
right after exp -- the softmax denominator (s -> 1/s) is computed concurrently
and only gates the tiny ctx_row copy.  The first and last batches are loaded
and scored in quarter-chunks (exp + ctx per chunk, denominator deferred) so
the pipeline head starts ~6us earlier and the tail overlaps the flood.

Sharding: data-parallel over batch, 16 batches per core on 8 cores; weights
replicated.  hidden_states is read from HBM exactly once (fp32), cast to fp16
during the DMA (SWDGE cast), and never transposed.

Pipeline design:
  - The y16 load flood starts immediately; ident/ht/wst ride the SWDGE queue
    AHEAD of the flood (they complete in ring order ~10us; anything on the
    sync ring during the flood takes 12..40us to land).  watt stays on the
    sync ring and is only casted right before the epilogue.
  - All u[b]/broadcast work happens on PE+ACT only, in fp16, interleaved with
    the batch loop.
  - softmax sum via PE ones-matmul; reciprocal on DVE right after the next
    batch's big mul (latency fully covered).
"""

import sys

import numpy as np

_TRN_REPO = "/opt/trn_rl_repo"
if _TRN_REPO not in sys.path:
    sys.path.insert(0, _TRN_REPO)

import concourse.bass as bass
import concourse.bacc as bacc
import concourse.tile as tile
from concourse import mybir
from concourse.bass_utils import run_bass_kernel_spmd

N_CORES = 8
B, T, H = 128, 2048, 256
NB = B // N_CORES  # batches per core
P = 128  # SBUF partitions
TT = T // P  # t-tiles per batch
OUT_D = 128
EXP_SHIFT = -50.0  # keeps exp() in fp32/bf16 range; cancels in the softmax ratio

NCH = 4  # first/last batches are loaded/scored in NCH chunks
CTT = TT // NCH
UPRE = 4  # u-chains emitted before the loop; chain b+UPRE emitted in iter b
N_ACT = 6  # t-tiles per full batch reduced on ACT instead of the DVE tree
N_DVE = TT - N_ACT

F32 = mybir.dt.float32
F16 = mybir.dt.float16
BF16 = mybir.dt.bfloat16


def _build_kernel(nc: bass.Bass, tc: "tile.TileContext", hidden, wst, watt, ident, out):
    add = mybir.AluOpType.add

    from contextlib import ExitStack

    with ExitStack() as ctx:
        const = ctx.enter_context(tc.tile_pool(name="const", bufs=1))
        ybufs = ctx.enter_context(tc.tile_pool(name="ybufs", bufs=10))
        sc = ctx.enter_context(tc.tile_pool(name="sc", bufs=3))
        psum_t = ctx.enter_context(tc.tile_pool(name="psum_t", bufs=2, space="PSUM"))
        psum_u = ctx.enter_context(tc.tile_pool(name="psum_u", bufs=2, space="PSUM"))
        psum_p = ctx.enter_context(tc.tile_pool(name="psum_p", bufs=1, space="PSUM"))

        # ---- constants (no DMA needed) -------------------------------------
        ones_row16 = const.tile([1, P], F16, tag="ones_row16")
        nc.vector.memset(ones_row16, 1.0)
        ones128 = const.tile([P, P], F32, tag="ones128")
        nc.vector.memset(ones128, 1.0)
        ones_col1 = const.tile([1, 1], F32, tag="ones_col1")
        nc.vector.memset(ones_col1, 1.0)
        shift_col = const.tile([P, 1], F32, tag="shift_col")
        nc.vector.memset(shift_col, EXP_SHIFT)

        # ---- setup DMAs ----------------------------------------------------
        ident_sb = const.tile([16, 16], F32, tag="ident")
        nc.gpsimd.dma_start(out=ident_sb, in_=ident[:, :])
        ht_sb = const.tile([NB, H], F32, tag="ht")
        nc.gpsimd.dma_start(out=ht_sb, in_=hidden[:, T - 1, :])
        wst_sb = const.tile([P, 2, H], F32, tag="wst")  # W_score^T as [k, kk, h]
        nc.gpsimd.dma_start(out=wst_sb, in_=wst.rearrange("(kk p) h -> p kk h", p=P))
        watt_sb = const.tile([P, 4, OUT_D], F32, tag="watt")  # W_att as [d, dd, j]
        nc.sync.dma_start(out=watt_sb, in_=watt.rearrange("(dd p) j -> p dd j", p=P))

        # ---- y16 load flood (SWDGE cast fp32->fp16), starts immediately ----
        CHUNKED = (0, NB - 1)
        ylist = {}
        ychunks = {}
        for k in range(NB):
            if k in CHUNKED:
                hk = hidden[k].rearrange("(p i) h -> p i h", i=TT)
                tiles = []
                for c in range(NCH):
                    yc = ybufs.tile([P, CTT, H], F16, tag="y16c", name=f"y16c_{k}_{c}")
                    nc.gpsimd.dma_start(out=yc, in_=hk[:, c * CTT : (c + 1) * CTT, :])
                    tiles.append(yc)
                ychunks[k] = tiles
            else:
                y = ybufs.tile([P, TT, H], F16, tag="y16", name=f"y16_{k}")
                nc.gpsimd.dma_start(
                    out=y, in_=hidden[k].rearrange("(p i) h -> p i h", i=TT)
                )
                ylist[k] = y

        # ---- h_t^T (fp16) and fp16 copy of W_score^T -----------------------
        htT16 = const.tile([P, 2, NB], F16, tag="htT16")  # h_t^T halves [k, half, b]
        for half in range(2):
            ps_tr = psum_t.tile([P, NB], F32, tag="ptmp", name=f"ps_tr{half}")
            nc.tensor.matmul(
                ps_tr,
                lhsT=ht_sb[:, half * P : (half + 1) * P],
                rhs=ident_sb,
                start=True,
                stop=True,
            )
            nc.scalar.copy(out=htT16[:, half, :], in_=ps_tr)
        wst16 = const.tile([P, 2, H], F16, tag="wst16")
        nc.scalar.copy(out=wst16, in_=wst_sb)

        # u[b] = h_t[b] @ W_score^T via M=1 fp16 matmuls; broadcast via a K=1
        # matmul.  No DMAs -> nothing contends with the flood.
        ubc_all = const.tile([P, NB, H], F16, tag="ubc_all")

        def emit_uchain(b):
            ps_ub = psum_u.tile([1, H], F32, tag="pub", name=f"pub{b}")
            for half in range(2):
                nc.tensor.matmul(
                    ps_ub,
                    lhsT=htT16[:, half, b : b + 1],
                    rhs=wst16[:, half, :],
                    start=(half == 0),
                    stop=(half == 1),
                )
            u16b = sc.tile([1, H], F16, tag="u16b", name=f"u16b{b}")
            nc.scalar.copy(out=u16b, in_=ps_ub)
            ps_ubc = psum_t.tile([P, H], F32, tag="ptmp", name=f"pubc{b}")
            nc.tensor.matmul(ps_ubc, lhsT=ones_row16, rhs=u16b, start=True, stop=True)
            nc.scalar.copy(out=ubc_all[:, b, :], in_=ps_ubc)

        for b in range(UPRE + 1):  # loop below starts at b=1, so chains 0..4 here
            emit_uchain(b)

        # ---- persistent PSUM accumulators for ctx^T ------------------------
        ctxT_ps = [
            psum_p.tile([P, NB], F32, tag=f"ctxT{j}", name=f"ctxT{j}")
            for j in range(2)
        ]

        state = {}  # batch -> dict of tiles needed by the deferred stages

        def ubc_rep(b, rep):
            ubc = ubc_all[:, b, :]
            return bass.AP(
                tensor=ubc.tensor,
                offset=ubc.offset,
                ap=[list(ubc.ap[0]), [0, rep], list(ubc.ap[1])],
            )

        def dve_tree(z, score_sl, nt, tag_sfx):
            z1 = sc.tile([P, nt, 128], F16, tag="z1" + tag_sfx)
            nc.vector.tensor_add(z1, z[:, :, 0:128], z[:, :, 128:256])
            z2 = sc.tile([P, nt, 64], F16, tag="z2" + tag_sfx)
            nc.vector.tensor_add(z2, z1[:, :, 0:64], z1[:, :, 64:128])
            nc.vector.tensor_reduce(
                out=score_sl, in_=z2, axis=mybir.AxisListType.X, op=add
            )

        def emit_exp(score_sl, p_sl, q_sl):
            nc.scalar.activation(
                out=p_sl,
                in_=score_sl,
                func=mybir.ActivationFunctionType.Exp,
                bias=shift_col,
                scale=1.0,
                accum_out=q_sl,
            )

        def emit_recip(b):
            rs = sc.tile([P, 1], F32, tag="rs", name=f"rs{b}")
            nc.vector.reciprocal(out=rs, in_=state[b]["s_ps"])
            state[b]["rs"] = rs

        def emit_ctx(b):
            p_t = state[b]["p"]
            ctx_ps = psum_t.tile([1, H], F32, tag="ptmp", name=f"ctx{b}")
            y16 = ylist.pop(b)
            for i in range(TT):
                nc.tensor.matmul(
                    ctx_ps,
                    lhsT=p_t[:, i : i + 1],
                    rhs=y16[:, i, :],
                    start=(i == 0),
                    stop=(i == TT - 1),
                )
            state[b]["ctx_ps"] = ctx_ps

        def emit_ctx_row(b):
            # normalization by 1/s happens here, on the [1, 256] row only
            ctx_row = sc.tile([1, H], F32, tag="ctx_row")
            nc.scalar.mul(
                out=ctx_row, in_=state[b]["ctx_ps"], mul=state[b]["rs"][0:1, :]
            )
            state[b]["ctx_row"] = ctx_row

        def emit_scatter(b):
            ctx_row = state[b]["ctx_row"]
            for j in range(2):
                nc.tensor.matmul(
                    ctxT_ps[j][:, b : b + 1],
                    lhsT=ctx_row[:, j * P : (j + 1) * P],
                    rhs=ones_col1,
                    start=True,
                    stop=True,
                )
            del state[b]

        def emit_chunked(b, after_first_mul=None):
            # quarter-chunk pipeline: mul/tree/exp/ctx per chunk; softmax sum
            # assembled at the end (bf16 p needs no pre-normalization)
            chunks = ychunks[b]
            score = sc.tile([P, TT], F32, tag="score", name=f"score{b}")
            p_t = sc.tile([P, TT], BF16, tag="p", name=f"p{b}")
            q4 = sc.tile([P, NCH], F32, tag="q4", name=f"q4_{b}")
            ctx_ps = psum_t.tile([1, H], F32, tag="ptmp", name=f"ctxc{b}")
            state[b] = {"p": p_t}
            for c in range(NCH):
                zc = sc.tile([P, CTT, H], F16, tag="zc")
                nc.vector.tensor_mul(zc, chunks[c], ubc_rep(b, CTT))
                if c == 0 and after_first_mul is not None:
                    after_first_mul()
                sl = slice(c * CTT, (c + 1) * CTT)
                dve_tree(zc, score[:, sl], CTT, "c")
                emit_exp(score[:, sl], p_t[:, sl], q4[:, c : c + 1])
                for i in range(c * CTT, (c + 1) * CTT):
                    nc.tensor.matmul(
                        ctx_ps,
                        lhsT=p_t[:, i : i + 1],
                        rhs=chunks[c][:, i % CTT, :],
                        start=(i == 0),
                        stop=(i == TT - 1),
                    )
            state[b]["ctx_ps"] = ctx_ps
            s4_ps = psum_u.tile([P, NCH], F32, tag="pub", name=f"s4_{b}")
            nc.tensor.matmul(s4_ps, lhsT=ones128, rhs=q4, start=True, stop=True)
            s_tot = sc.tile([P, 1], F32, tag="rs", name=f"stot{b}")
            nc.vector.tensor_reduce(
                out=s_tot, in_=s4_ps, axis=mybir.AxisListType.X, op=add
            )
            rs = sc.tile([P, 1], F32, tag="rs", name=f"rsc{b}")
            nc.vector.reciprocal(out=rs, in_=s_tot)
            state[b]["rs"] = rs

        # ---- batch 0: chunked so the pipeline head starts ~6us earlier -----
        emit_chunked(0)

        # ---- full batches 1..14 --------------------------------------------
        for b in range(1, NB - 1):
            if b + UPRE < NB:
                emit_uchain(b + UPRE)
            z = sc.tile([P, TT, H], F16, tag="z")
            nc.vector.tensor_mul(z, ylist[b], ubc_rep(b, TT))
            if b - 1 not in CHUNKED:
                emit_ctx(b - 1)
                emit_recip(b - 1)
            score = sc.tile([P, TT], F32, tag="score")
            dve_tree(z[:, 0:N_DVE, :], score[:, 0:N_DVE], N_DVE, "")
            for i in range(N_DVE, TT):
                zdump = sc.tile([P, H], F16, tag="zdump")
                nc.scalar.activation(
                    out=zdump,
                    in_=z[:, i, :],
                    func=mybir.ActivationFunctionType.Copy,
                    accum_out=score[:, i : i + 1],
                )
            p_t = sc.tile([P, TT], BF16, tag="p")
            q = sc.tile([P, 1], F32, tag="q")
            state[b] = {"p": p_t}
            emit_exp(score, p_t, q)
            s_ps = psum_u.tile([P, 1], F32, tag="pub", name=f"s{b}")
            nc.tensor.matmul(s_ps, lhsT=ones128, rhs=q, start=True, stop=True)
            state[b]["s_ps"] = s_ps
            emit_ctx_row(b - 1)
            emit_scatter(b - 1)

        # ---- last batch: chunked (tail overlaps the flood) -----------------
        def _finish_b14():
            emit_ctx(NB - 2)
            emit_recip(NB - 2)
            emit_ctx_row(NB - 2)
            emit_scatter(NB - 2)

        emit_chunked(NB - 1, after_first_mul=_finish_b14)
        emit_ctx_row(NB - 1)
        emit_scatter(NB - 1)

        # ---- finalize: concat with h_t, @W_att, tanh -----------------------
        # watt16 cast sits here so its wait on the (slow, sync-ring) watt DMA
        # never head-of-line blocks the per-batch ACT stream
        watt16 = const.tile([P, 4, OUT_D], F16, tag="watt16")
        nc.scalar.copy(out=watt16, in_=watt_sb)
        preT = sc.tile([P, 2, NB], F16, tag="preT")
        for j in range(2):
            nc.scalar.copy(out=preT[:, j, :], in_=ctxT_ps[j])

        out_ps = psum_t.tile([NB, OUT_D], F32, tag="ptmp")
        for dd in range(4):
            lhsT = preT[:, dd, :] if dd < 2 else htT16[:, dd - 2, :]
            nc.tensor.matmul(
                out_ps,
                lhsT=lhsT,
                rhs=watt16[:, dd, :],
                start=(dd == 0),
                stop=(dd == 3),
            )
        out_sb = sc.tile([NB, OUT_D], F32, tag="out_sb")
        nc.scalar.activation(
            out=out_sb, in_=out_ps, func=mybir.ActivationFunctionType.Tanh
        )
        nc.sync.dma_start(out=out[:, :], in_=out_sb)


_NC_CACHE = {}


def _get_nc():
    if "nc" not in _NC_CACHE:
        nc = bacc.Bacc("TRN2", target_bir_lowering=False, debug=False)
        hidden = nc.declare_dram_parameter("hidden", [NB, T, H], F32, isOutput=False)
        wst = nc.declare_dram_parameter("w_score_t", [H, H], F32, isOutput=False)
        watt = nc.declare_dram_parameter("w_att", [2 * H, OUT_D], F32, isOutput=False)
        ident = nc.declare_dram_parameter("ident16", [16, 16], F32, isOutput=False)
        out = nc.declare_dram_parameter("out", [NB, OUT_D], F32, isOutput=True)
        with tile.TileContext(nc) as tc:
            _build_kernel(nc, tc, hidden, wst, watt, ident, out)
        nc.compile()
        _NC_CACHE["nc"] = nc
    return _NC_CACHE["nc"]


def _run(hidden_states, W_score, W_att, trace=False, trace_kwargs=None):
    hidden_states = np.ascontiguousarray(np.asarray(hidden_states, dtype=np.float32))
    W_score = np.asarray(W_score, dtype=np.float32)
    W_att = np.ascontiguousarray(np.asarray(W_att, dtype=np.float32))
    wst = np.ascontiguousarray(W_score.T)
    ident = np.eye(16, dtype=np.float32)

    nc = _get_nc()
    in_maps = []
    for c in range(N_CORES):
        in_maps.append(
            {
                "hidden": hidden_states[c * NB : (c + 1) * NB],
                "w_score_t": wst,
                "w_att": W_att,
                "ident16": ident,
            }
        )
    kwargs = {}
    if trace:
        kwargs["trace"] = True
        if trace_kwargs:
            kwargs.update(trace_kwargs)
    res = run_bass_kernel_spmd(nc, in_maps, list(range(N_CORES)), **kwargs)
    out = np.concatenate([res.results[c]["out"] for c in range(N_CORES)], axis=0)
    return out, res


def kernel(hidden_states, W_score, W_att):
    out, _ = _run(hidden_states, W_score, W_att, trace=False)
    return out


# revision 16
# speedup vs baseline: 1.1798x; 1.0257x over previous
"""Trainium2 Bass kernel for nn_Attention (pooling attention).

Math (per batch b):
    u[b]     = W_score @ h_t[b]            (score = (hidden @ W_score) . h_t
                                            collapses to hidden . (W_score @ h_t))
    score[t] = hidden[b,t,:] . u[b]        (DVE fp16 mul + split reduction:
                                            10 t-tiles via DVE pairwise tree,
                                            6 t-tiles via ACT copy-with-accum)
    p[t]     = exp(score[t] - 50)          (ScalarE -> bf16, fused accum -> q)
    s        = sum_t p[t]                  (PE ones-matmul over q)
    ctx      = (sum_t p[t] * hidden[b,t,:]) / s
               (PE: bf16 p column as 1-col stationary vs fp16 y; the 1/s
                normalization folds into the PSUM->SBUF copy via ACT scale)
    out[b]   = tanh([ctx, h_t[b]] @ W_att)

bf16 p is overflow-safe (fp32-range exponent), so the context matmuls start
right after exp -- the softmax denominator (s -> 1/s) is computed concurrently
and only gates the tiny ctx_row copy.  The first and last batches are loaded
and scored in quarter-chunks (exp + ctx per chunk, denominator deferred) so
the pipeline head starts ~6us earlier and the tail overlaps the flood.

Sharding: data-parallel over batch, 16 batches per core on 8 cores; weights
replicated.  hidden_states is read from HBM exactly once (fp32), cast to fp16
during the DMA (SWDGE cast), and never transposed.

Pipeline design:
  - The y16 load flood starts immediately; ident/ht/wst ride the SWDGE queue
    AHEAD of the flood (they complete in ring order ~10us; anything on the
    sync ring during the flood takes 12..40us to land).  watt stays on the
    sync ring and is only casted right before the epilogue.
  - All u[b]/broadcast work happens on PE+ACT only, in fp16, interleaved with
    the batch loop.
  - softmax sum via PE ones-matmul; reciprocal on DVE right after the next
    batch's big mul (latency fully covered).
"""

import sys

import numpy as np

_TRN_REPO = "/opt/trn_rl_repo"
if _TRN_REPO not in sys.path:
    sys.path.insert(0, _TRN_REPO)

import concourse.bass as bass
import concourse.bacc as bacc
import concourse.tile as tile
from concourse import mybir
from concourse.bass_utils import run_bass_kernel_spmd

N_CORES = 8
B, T, H = 128, 2048, 256
NB = B // N_CORES  # batches per core
P = 128  # SBUF partitions
TT = T // P  # t-tiles per batch
OUT_D = 128
EXP_SHIFT = -50.0  # keeps exp() in fp32/bf16 range; cancels in the softmax ratio

NCH = 4  # first/last batches are loaded/scored in NCH chunks
CTT = TT // NCH
UPRE = 4  # u-chains emitted before the loop; chain b+UPRE emitted in iter b

F32 = mybir.dt.float32
F16 = mybir.dt.float16
BF16 = mybir.dt.bfloat16


def _build_kernel(nc: bass.Bass, tc: "tile.TileContext", hidden, wst, watt, ident, out):
    add = mybir.AluOpType.add

    from contextlib import ExitStack

    with ExitStack() as ctx:
        const = ctx.enter_context(tc.tile_pool(name="const", bufs=1))
        ybufs = ctx.enter_context(tc.tile_pool(name="ybufs", bufs=10))
        sc = ctx.enter_context(tc.tile_pool(name="sc", bufs=3))
        psum_t = ctx.enter_context(tc.tile_pool(name="psum_t", bufs=2, space="PSUM"))
        psum_u = ctx.enter_context(tc.tile_pool(name="psum_u", bufs=2, space="PSUM"))
        psum_p = ctx.enter_context(tc.tile_pool(name="psum_p", bufs=1, space="PSUM"))

        # ---- constants (no DMA needed) -------------------------------------
        ones_row16 = const.tile([1, P], F16, tag="ones_row16")
        nc.vector.memset(ones_row16, 1.0)
        ones128 = const.tile([P, P], F32, tag="ones128")
        nc.vector.memset(ones128, 1.0)
        ones_col1 = const.tile([1, 1], F32, tag="ones_col1")
        nc.vector.memset(ones_col1, 1.0)
        shift_col = const.tile([P, 1], F32, tag="shift_col")
        nc.vector.memset(shift_col, EXP_SHIFT)

        # ---- setup DMAs ----------------------------------------------------
        ident_sb = const.tile([16, 16], F32, tag="ident")
        nc.gpsimd.dma_start(out=ident_sb, in_=ident[:, :])
        ht_sb = const.tile([NB, H], F32, tag="ht")
        nc.gpsimd.dma_start(out=ht_sb, in_=hidden[:, T - 1, :])
        wst_sb = const.tile([P, 2, H], F32, tag="wst")  # W_score^T as [k, kk, h]
        nc.gpsimd.dma_start(out=wst_sb, in_=wst.rearrange("(kk p) h -> p kk h", p=P))
        watt_sb = const.tile([P, 4, OUT_D], F32, tag="watt")  # W_att as [d, dd, j]
        nc.sync.dma_start(out=watt_sb, in_=watt.rearrange("(dd p) j -> p dd j", p=P))

        # ---- y16 load flood (SWDGE cast fp32->fp16), starts immediately ----
        CHUNKED = (0, NB - 1)
        ylist = {}
        ychunks = {}
        for k in range(NB):
            if k in CHUNKED:
                hk = hidden[k].rearrange("(p i) h -> p i h", i=TT)
                tiles = []
                for c in range(NCH):
                    yc = ybufs.tile([P, CTT, H], F16, tag="y16c", name=f"y16c_{k}_{c}")
                    nc.gpsimd.dma_start(out=yc, in_=hk[:, c * CTT : (c + 1) * CTT, :])
                    tiles.append(yc)
                ychunks[k] = tiles
            else:
                y = ybufs.tile([P, TT, H], F16, tag="y16", name=f"y16_{k}")
                nc.gpsimd.dma_start(
                    out=y, in_=hidden[k].rearrange("(p i) h -> p i h", i=TT)
                )
                ylist[k] = y

        # ---- h_t^T (fp16) and fp16 copy of W_score^T -----------------------
        htT16 = const.tile([P, 2, NB], F16, tag="htT16")  # h_t^T halves [k, half, b]
        for half in range(2):
            ps_tr = psum_t.tile([P, NB], F32, tag="ptmp", name=f"ps_tr{half}")
            nc.tensor.matmul(
                ps_tr,
                lhsT=ht_sb[:, half * P : (half + 1) * P],
                rhs=ident_sb,
                start=True,
                stop=True,
            )
            nc.scalar.copy(out=htT16[:, half, :], in_=ps_tr)
        wst16 = const.tile([P, 2, H], F16, tag="wst16")
        nc.scalar.copy(out=wst16, in_=wst_sb)

        # u[b] = h_t[b] @ W_score^T via M=1 fp16 matmuls; broadcast via a K=1
        # matmul.  No DMAs -> nothing contends with the flood.
        ubc_all = const.tile([P, NB, H], F16, tag="ubc_all")

        def emit_uchain(b):
            ps_ub = psum_u.tile([1, H], F32, tag="pub", name=f"pub{b}")
            for half in range(2):
                nc.tensor.matmul(
                    ps_ub,
                    lhsT=htT16[:, half, b : b + 1],
                    rhs=wst16[:, half, :],
                    start=(half == 0),
                    stop=(half == 1),
                )
            u16b = sc.tile([1, H], F16, tag="u16b", name=f"u16b{b}")
            nc.scalar.copy(out=u16b, in_=ps_ub)
            ps_ubc = psum_t.tile([P, H], F32, tag="ptmp", name=f"pubc{b}")
            nc.tensor.matmul(ps_ubc, lhsT=ones_row16, rhs=u16b, start=True, stop=True)
            nc.scalar.copy(out=ubc_all[:, b, :], in_=ps_ubc)

        for b in range(UPRE + 1):  # loop below starts at b=1, so chains 0..4 here
            emit_uchain(b)

        # ---- persistent PSUM accumulators for ctx^T ------------------------
        ctxT_ps = [
            psum_p.tile([P, NB], F32, tag=f"ctxT{j}", name=f"ctxT{j}")
            for j in range(2)
        ]

        state = {}  # batch -> dict of tiles needed by the deferred stages

        def ubc_rep(b, rep):
            ubc = ubc_all[:, b, :]
            return bass.AP(
                tensor=ubc.tensor,
                offset=ubc.offset,
                ap=[list(ubc.ap[0]), [0, rep], list(ubc.ap[1])],
            )

        def dve_tree(z, score_sl, nt, tag_sfx):
            z1 = sc.tile([P, nt, 128], F16, tag="z1" + tag_sfx)
            nc.vector.tensor_add(z1, z[:, :, 0:128], z[:, :, 128:256])
            z2 = sc.tile([P, nt, 64], F16, tag="z2" + tag_sfx)
            nc.vector.tensor_add(z2, z1[:, :, 0:64], z1[:, :, 64:128])
            # fp16 score: keeps the reduce in the DVE 2x perf mode; |score|<~90
            # and the softmax ratio tolerates the ~0.05 rounding (verified in
            # the rel-err gate)
            with nc.allow_low_precision(reason="fp16 softmax scores"):
                nc.vector.tensor_reduce(
                    out=score_sl, in_=z2, axis=mybir.AxisListType.X, op=add
                )

        def emit_exp(score_sl, p_sl, q_sl):
            nc.scalar.activation(
                out=p_sl,
                in_=score_sl,
                func=mybir.ActivationFunctionType.Exp,
                bias=shift_col,
                scale=1.0,
                accum_out=q_sl,
            )

        def emit_recip(b):
            rs = sc.tile([P, 1], F32, tag="rs", name=f"rs{b}")
            nc.vector.reciprocal(out=rs, in_=state[b]["s_ps"])
            state[b]["rs"] = rs

        def emit_ctx(b):
            p_t = state[b]["p"]
            ctx_ps = psum_t.tile([1, H], F32, tag="ptmp", name=f"ctx{b}")
            y16 = ylist.pop(b)
            for i in range(TT):
                nc.tensor.matmul(
                    ctx_ps,
                    lhsT=p_t[:, i : i + 1],
                    rhs=y16[:, i, :],
                    start=(i == 0),
                    stop=(i == TT - 1),
                )
            state[b]["ctx_ps"] = ctx_ps

        def emit_ctx_row(b):
            # normalization by 1/s happens here, on the [1, 256] row only
            ctx_row = sc.tile([1, H], F32, tag="ctx_row")
            nc.scalar.mul(
                out=ctx_row, in_=state[b]["ctx_ps"], mul=state[b]["rs"][0:1, :]
            )
            state[b]["ctx_row"] = ctx_row

        def emit_scatter(b):
            ctx_row = state[b]["ctx_row"]
            for j in range(2):
                nc.tensor.matmul(
                    ctxT_ps[j][:, b : b + 1],
                    lhsT=ctx_row[:, j * P : (j + 1) * P],
                    rhs=ones_col1,
                    start=True,
                    stop=True,
                )
            del state[b]

        def emit_chunked(b, after_first_mul=None):
            # quarter-chunk pipeline: mul/tree/exp/ctx per chunk; softmax sum
            # assembled at the end (bf16 p needs no pre-normalization)
            chunks = ychunks[b]
            score = sc.tile([P, TT], F16, tag="score", name=f"score{b}")
            p_t = sc.tile([P, TT], BF16, tag="p", name=f"p{b}")
            q4 = sc.tile([P, NCH], F32, tag="q4", name=f"q4_{b}")
            ctx_ps = psum_t.tile([1, H], F32, tag="ptmp", name=f"ctxc{b}")
            state[b] = {"p": p_t}
            for c in range(NCH):
                zc = sc.tile([P, CTT, H], F16, tag="zc")
                nc.vector.tensor_mul(zc, chunks[c], ubc_rep(b, CTT))
                if c == 0 and after_first_mul is not None:
                    after_first_mul()
                sl = slice(c * CTT, (c + 1) * CTT)
                dve_tree(zc, score[:, sl], CTT, "c")
                emit_exp(score[:, sl], p_t[:, sl], q4[:, c : c + 1])
                for i in range(c * CTT, (c + 1) * CTT):
                    nc.tensor.matmul(
                        ctx_ps,
                        lhsT=p_t[:, i : i + 1],
                        rhs=chunks[c][:, i % CTT, :],
                        start=(i == 0),
                        stop=(i == TT - 1),
                    )
            state[b]["ctx_ps"] = ctx_ps
            s4_ps = psum_u.tile([P, NCH], F32, tag="pub", name=f"s4_{b}")
            nc.tensor.matmul(s4_ps, lhsT=ones128, rhs=q4, start=True, stop=True)
            s_tot = sc.tile([P, 1], F32, tag="rs", name=f"stot{b}")
            nc.vector.tensor_reduce(
                out=s_tot, in_=s4_ps, axis=mybir.AxisListType.X, op=add
            )
            rs = sc.tile([P, 1], F32, tag="rs", name=f"rsc{b}")
            nc.vector.reciprocal(out=rs, in_=s_tot)
            state[b]["rs"] = rs

        # ---- batch 0: chunked so the pipeline head starts ~6us earlier -----
        emit_chunked(0)

        # ---- full batches 1..14 --------------------------------------------
        for b in range(1, NB - 1):
            if b + UPRE < NB:
                emit_uchain(b + UPRE)
            z = sc.tile([P, TT, H], F16, tag="z")
            nc.vector.tensor_mul(z, ylist[b], ubc_rep(b, TT))
            if b - 1 not in CHUNKED:
                emit_ctx(b - 1)
                emit_recip(b - 1)
            score = sc.tile([P, TT], F16, tag="score")
            dve_tree(z, score, TT, "")
            p_t = sc.tile([P, TT], BF16, tag="p")
            q = sc.tile([P, 1], F32, tag="q")
            state[b] = {"p": p_t}
            emit_exp(score, p_t, q)
            s_ps = psum_u.tile([P, 1], F32, tag="pub", name=f"s{b}")
            nc.tensor.matmul(s_ps, lhsT=ones128, rhs=q, start=True, stop=True)
            state[b]["s_ps"] = s_ps
            emit_ctx_row(b - 1)
            emit_scatter(b - 1)

        # ---- last batch: chunked (tail overlaps the flood) -----------------
        def _finish_b14():
            emit_ctx(NB - 2)
            emit_recip(NB - 2)
            emit_ctx_row(NB - 2)
            emit_scatter(NB - 2)

        emit_chunked(NB - 1, after_first_mul=_finish_b14)
        emit_ctx_row(NB - 1)
        emit_scatter(NB - 1)

        # ---- finalize: concat with h_t, @W_att, tanh -----------------------
        # watt16 cast sits here so its wait on the (slow, sync-ring) watt DMA
        # never head-of-line blocks the per-batch ACT stream
        watt16 = const.tile([P, 4, OUT_D], F16, tag="watt16")
        nc.scalar.copy(out=watt16, in_=watt_sb)
        preT = sc.tile([P, 2, NB], F16, tag="preT")
        for j in range(2):
            nc.scalar.copy(out=preT[:, j, :], in_=ctxT_ps[j])

        out_ps = psum_t.tile([NB, OUT_D], F32, tag="ptmp")
        for dd in range(4):
            lhsT = preT[:, dd, :] if dd < 2 else htT16[:, dd - 2, :]
            nc.tensor.matmul(
                out_ps,
                lhsT=lhsT,
                rhs=watt16[:, dd, :],
                start=(dd == 0),
                stop=(dd == 3),
            )
        out_sb = sc.tile([NB, OUT_D], F32, tag="out_sb")
        nc.scalar.activation(
            out=out_sb, in_=out_ps, func=mybir.ActivationFunctionType.Tanh
        )
        nc.sync.dma_start(out=out[:, :], in_=out_sb)


_NC_CACHE = {}


def _get_nc():
    if "nc" not in _NC_CACHE:
        nc = bacc.Bacc("TRN2", target_bir_lowering=False, debug=False)
        hidden = nc.declare_dram_parameter("hidden", [NB, T, H], F32, isOutput=False)
        wst = nc.declare_dram_parameter("w_score_t", [H, H], F32, isOutput=False)
        watt = nc.declare_dram_parameter("w_att", [2 * H, OUT_D], F32, isOutput=False)
        ident = nc.declare_dram_parameter("ident16", [16, 16], F32, isOutput=False)
        out = nc.declare_dram_parameter("out", [NB, OUT_D], F32, isOutput=True)
        with tile.TileContext(nc) as tc:
            _build_kernel(nc, tc, hidden, wst, watt, ident, out)
        nc.compile()
        _NC_CACHE["nc"] = nc
    return _NC_CACHE["nc"]


def _run(hidden_states, W_score, W_att, trace=False, trace_kwargs=None):
    hidden_states = np.ascontiguousarray(np.asarray(hidden_states, dtype=np.float32))
    W_score = np.asarray(W_score, dtype=np.float32)
    W_att = np.ascontiguousarray(np.asarray(W_att, dtype=np.float32))
    wst = np.ascontiguousarray(W_score.T)
    ident = np.eye(16, dtype=np.float32)

    nc = _get_nc()
    in_maps = []
    for c in range(N_CORES):
        in_maps.append(
            {
                "hidden": hidden_states[c * NB : (c + 1) * NB],
                "w_score_t": wst,
                "w_att": W_att,
                "ident16": ident,
            }
        )
    kwargs = {}
    if trace:
        kwargs["trace"] = True
        if trace_kwargs:
            kwargs.update(trace_kwargs)
    res = run_bass_kernel_spmd(nc, in_maps, list(range(N_CORES)), **kwargs)
    out = np.concatenate([res.results[c]["out"] for c in range(N_CORES)], axis=0)
    return out, res


def kernel(hidden_states, W_score, W_att):
    out, _ = _run(hidden_states, W_score, W_att, trace=False)
    return out
